# revision 17
# baseline (speedup 1.0000x reference)
"""Causal single-head attention on 8 Trainium2 NeuronCores.

Reference computation (per batch b of 16):
    q = x @ Wq; k = x @ Wk; v = x @ Wv        # x [2048, 512], W* [512, 64]
    out = softmax_causal(q @ k.T / 8) @ v     # out [2048, 64]

Sharding: data-parallel over batch, 2 batches per core, weights replicated.

Per-core kernel (batch-local b in {0,1}), all matmuls in float32r
(TF32-like, full PE rate at N>=256):
  - host supplies xT = x[b].T so the D-contraction sits on partitions
  - qT/kT: psum[0:64]=qT, psum[64:128]=kT via packed lhsT [Wq|Wk]
  - kT copied to partitions 0-63 via SBUF->SBUF DMA (matmul operands
    must share partitions)
  - vT via Wv-stationary matmuls, then PE-transposed to v natural and
    packed as v1[., j, .] = [v_j | 1] (the ones column makes the PV
    matmul emit the softmax denominator for free)
  - scores TRANSPOSED: ST[k, q] = kT_j.T @ qT_Q -> psum, so softmax's
    denominator is a partition-dim sum that the PV matmul computes,
    and p~ = exp(ST) feeds the PV matmul with no transpose
  - exp on ACT straight out of psum in [128, 1024] chunks (2 k-blocks)
  - causal: k-blocks above the diagonal skipped; diagonal blocks get a
    triangular mask multiply and suffix-sliced matmuls
  - oT[65, 512] accumulates [v|1].T @ p~ over k-blocks in psum; row 64
    is the denominator l; out = oT[0:64] * broadcast(1/l) where the
    broadcast across partitions is a K=1 matmul against a ones column
  - output written transposed [2, 64, 2048]; host transposes back
"""

import sys

sys.path.insert(0, "/opt/trn_rl_repo")

import numpy as np

B, T, D, HD = 16, 2048, 512, 64
NCORES = 8
BPC = B // NCORES          # batches per core
NQ = T // 512              # 512-wide q chunks per batch
NJ = T // 128              # 128-wide k blocks per batch
ND = D // 128              # 128-deep contraction tiles

_cache = {}


def _build_nc():
    import concourse.bacc as bacc
    import concourse.mybir as mybir
    import concourse.tile as tile

    F32 = mybir.dt.float32
    F32R = mybir.dt.float32r
    AF = mybir.ActivationFunctionType

    nc = bacc.Bacc("TRN2", target_bir_lowering=False, debug=False)

    xt_d = nc.dram_tensor("xt", [BPC, D, T], F32, kind="ExternalInput")
    wqk_d = nc.dram_tensor("wqk", [ND, 128, 128], F32, kind="ExternalInput")
    wv_d = nc.dram_tensor("wv", [ND, 128, HD], F32, kind="ExternalInput")
    ident_d = nc.dram_tensor("ident", [64, 64], F32, kind="ExternalInput")
    mask_d = nc.dram_tensor("mask", [128, 128], F32, kind="ExternalInput")
    zeros_d = nc.dram_tensor("zeros", [64, T], F32, kind="ExternalInput")
    onescol_d = nc.dram_tensor("onescol", [128, NJ], F32, kind="ExternalInput")
    ot_d = nc.dram_tensor("ot", [BPC, HD, T], F32, kind="ExternalOutput")

    with tile.TileContext(nc) as tc:
        with (
            tc.tile_pool(name="const", bufs=1) as cpool,
            tc.tile_pool(name="xt", bufs=1) as xtpool,
            tc.tile_pool(name="qk", bufs=2) as qkpool,
            tc.tile_pool(name="klo", bufs=2) as klopool,
            tc.tile_pool(name="vt", bufs=2) as vtpool,
            tc.tile_pool(name="v1", bufs=2) as v1pool,
            tc.tile_pool(name="pt", bufs=3) as ptpool,
            tc.tile_pool(name="outp", bufs=2) as outpool,
            tc.tile_pool(name="rec", bufs=2) as recpool,
            tc.tile_pool(name="st", bufs=2, space="PSUM") as stpool,
            tc.tile_pool(name="otp", bufs=2, space="PSUM") as otpool,
            tc.tile_pool(name="aux", bufs=2, space="PSUM") as auxpool,
        ):
            # ---- per-batch persistent tiles; zero K-pad rows for klo,
            # split across DMA queues and issued first (single-queue DMA is
            # ~25 GB/s; latency matters) ----
            _qkpools = (qkpool, klopool, v1pool, vtpool)
            PRE = {}
            for b in range(BPC):
                PRE[b] = (
                    qkpool.tile([128, T], F32R, tag="qk", name=f"qk{b}"),
                    klopool.tile([128, T], F32R, tag="klo", name=f"klo{b}"),
                    v1pool.tile([128, NJ, HD + 1], F32R, tag="v1", name=f"v1{b}"),
                    vtpool.tile([64, T], F32, tag="vt", name=f"vt{b}"),
                )
            for b in range(BPC):
                for z8 in range(8):
                    nc.sync.dma_start(
                        PRE[b][1][64 + 8 * z8:64 + 8 * (z8 + 1), :],
                        zeros_d[8 * z8:8 * (z8 + 1), :].bitcast(F32R),
                    )

            # ---- constants / weights ----

            ident = cpool.tile([64, 64], F32, tag="ident")
            nc.sync.dma_start(ident[:], ident_d[:])
            mask = cpool.tile([128, 128], F32, tag="mask")
            nc.sync.dma_start(mask[:], mask_d[:])
            onescol = cpool.tile([128, NJ], F32, tag="onescol")
            nc.sync.dma_start(onescol[:], onescol_d[:])
            wqk = []
            for d in range(ND):
                w = cpool.tile([128, 128], F32R, tag=f"wqk{d}", name=f"wqk{d}")
                nc.sync.dma_start(w[:], wqk_d[d].bitcast(F32R))
                wqk.append(w)
            wv = []
            for d in range(ND):
                w = cpool.tile([128, HD], F32R, tag=f"wv{d}", name=f"wv{d}")
                nc.sync.dma_start(w[:], wv_d[d].bitcast(F32R))
                wv.append(w)

            # warm the exp table set on ACT while projections run
            scratch = cpool.tile([1, 1], F32, tag="scratch")
            nc.scalar.activation(scratch[:], mask[0:1, 0:1], AF.Exp)


            xts = {}   # (b, d) -> tile [128, T] F32R
            qks, klos, v1s, vts = {}, {}, {}, {}
            for b in range(BPC):
                qks[b], klos[b], v1s[b], vts[b] = PRE[b]
                nc.vector.tensor_copy(
                    v1s[b][:, :, HD:HD + 1],
                    onescol[:].rearrange("p (a c) -> p a c", c=1),
                )

            def emit_xt_dmas(b, Q):
                for d in range(ND):
                    if (b, d) not in xts:
                        xts[(b, d)] = xtpool.tile(
                            [128, T], F32R, tag=f"xt{b}{d}", name=f"xt{b}{d}"
                        )
                    s = slice(512 * Q, 512 * (Q + 1))
                    for r4 in range(4):
                        nc.sync.dma_start(
                            xts[(b, d)][32 * r4:32 * (r4 + 1), s],
                            xt_d[b, 128 * d + 32 * r4:128 * d + 32 * (r4 + 1),
                                 s].bitcast(F32R),
                        )

            def emit_proj_q(b, Q):
                """Everything attention chunk (b, Q) will need from tokens
                [512Q, 512Q+512): qT/kT, k shifted+zero-padded, v transposed."""
                s = slice(512 * Q, 512 * (Q + 1))
                qk, klo, v1, vt = qks[b], klos[b], v1s[b], vts[b]

                p = auxpool.tile([128, 512], F32, tag="aux", name="pqk")
                for d in range(ND):
                    nc.tensor.matmul(
                        p[:], wqk[d][:], xts[(b, d)][:, s],
                        start=(d == 0), stop=(d == ND - 1),
                    )
                nc.vector.tensor_copy(qk[:, s], p[:])
                # kT to partitions 0:64 (rows 64:128 are the zero K-pad);
                # partition-split 4 ways so the shift spreads across queues
                for p4 in range(4):
                    nc.sync.dma_start(
                        klo[16 * p4:16 * (p4 + 1), s],
                        qk[64 + 16 * p4:64 + 16 * (p4 + 1), s],
                    )

                pv_ = auxpool.tile([64, 512], F32, tag="aux", name="pvt")
                for d in range(ND):
                    nc.tensor.matmul(
                        pv_[:], wv[d][:], xts[(b, d)][:, s],
                        start=(d == 0), stop=(d == ND - 1),
                    )
                nc.vector.tensor_copy(vt[:, s], pv_[:])
                for t2 in range(2 * Q, 2 * Q + 2):
                    p2 = auxpool.tile([128, 128], F32, tag="aux", name="ptr")
                    for tt in range(2):
                        nc.tensor.transpose(
                            p2[:, 64 * tt:64 * (tt + 1)],
                            vt[:, 128 * (2 * t2 + tt):128 * (2 * t2 + tt + 1)],
                            ident[:],
                        )
                    nc.vector.tensor_copy(
                        v1[:, 2 * t2:2 * t2 + 2, 0:HD],
                        p2[:].rearrange("p (a c) -> p a c", a=2),
                    )

            def emit_attn_q(b, Q):
                """One query chunk: all causal k-blocks, paired into
                [128,1024] psum chunks; PV skewed one chunk behind ST."""
                qk, klo, v1 = qks[b], klos[b], v1s[b]
                pot = otpool.tile([65, 512], F32, tag="ot", name="pot")
                njb = 4 * (Q + 1)          # causal k-blocks for this chunk
                jlast = njb - 1
                chunks = [(2 * g, 2 * g + 1) for g in range(njb // 2)]
                pending = None

                def emit_pv(p_tile, js):
                    for idx, j in enumerate(js):
                        w0 = 128 * (j - 4 * Q) if j >= 4 * Q else 0
                        nc.tensor.matmul(
                            pot[:, w0:512],
                            v1[:, j, :],
                            p_tile[:, 512 * idx + w0:512 * (idx + 1)],
                            start=(j == 0),
                            stop=(j == jlast),
                        )

                for js in chunks:
                    pst = stpool.tile([128, 1024], F32, tag="st", name="pst")
                    for idx, j in enumerate(js):
                        w0 = 128 * (j - 4 * Q) if j >= 4 * Q else 0
                        nc.tensor.matmul(
                            pst[:, 512 * idx + w0:512 * (idx + 1)],
                            klo[:, 128 * j:128 * (j + 1)],
                            qk[:, 512 * Q + w0:512 * (Q + 1)],
                            start=True, stop=True,
                        )
                    ptil = ptpool.tile([128, 1024], F32R, tag="pt", name="ptil")
                    nc.scalar.activation(
                        ptil[:], pst[:], AF.Exp, scale=1.0 / np.sqrt(HD)
                    )
                    for idx, j in enumerate(js):
                        if j >= 4 * Q:  # diagonal block: triangular mask
                            w0 = 128 * (j - 4 * Q)
                            win = slice(512 * idx + w0, 512 * idx + w0 + 128)
                            nc.vector.tensor_mul(
                                ptil[:, win], ptil[:, win].bitcast(F32), mask[:]
                            )
                    if pending is not None:
                        emit_pv(*pending)
                    pending = (ptil, js)
                emit_pv(*pending)

                # normalize: out = oT[0:64] * broadcast(1 / l), PE-free
                lsb = recpool.tile([1, 512], F32, tag="lsb", name="lsb")
                nc.vector.tensor_copy(lsb[:], pot[64:65, :])
                rsb = recpool.tile([1, 512], F32, tag="rsb", name="rsb")
                nc.vector.reciprocal_approx_fast(rsb[:], lsb[:])
                rcb = outpool.tile([HD, 512], F32, tag="rcb", name="rcb")
                nc.gpsimd.partition_broadcast(rcb[:], rsb[:])
                osb = outpool.tile([HD, 512], F32, tag="out", name="osb")
                nc.vector.tensor_mul(osb[:], pot[0:HD, :], rcb[:])
                nc.sync.dma_start(ot_d[0 if b == 0 else 1, :, 512 * Q:512 * (Q + 1)], osb[:])

            # ---- emission schedule: projections feed attention per-Q ----
            for Q in range(NQ):
                emit_xt_dmas(0, Q)
            emit_proj_q(0, 0)
            emit_attn_q(0, 0)
            emit_proj_q(0, 1)
            emit_attn_q(0, 1)
            emit_proj_q(0, 2)
            for Q in range(NQ):
                emit_xt_dmas(1, Q)
            emit_attn_q(0, 2)
            emit_proj_q(0, 3)
            emit_attn_q(0, 3)
            emit_proj_q(1, 0)
            emit_attn_q(1, 0)
            emit_proj_q(1, 1)
            emit_attn_q(1, 1)
            emit_proj_q(1, 2)
            emit_attn_q(1, 2)
            emit_proj_q(1, 3)
            emit_attn_q(1, 3)

    nc.compile()
    return nc


def _get_nc():
    if "nc" not in _cache:
        _cache["nc"] = _build_nc()
    return _cache["nc"]


def kernel(x, Wq, Wk, Wv, _trace=False, _trace_kwargs=None):
    from concourse.bass_utils import run_bass_kernel_spmd

    x = np.asarray(x, dtype=np.float32)
    Wq = np.asarray(Wq, dtype=np.float32)
    Wk = np.asarray(Wk, dtype=np.float32)
    Wv = np.asarray(Wv, dtype=np.float32)

    nc = _get_nc()

    wqk = np.ascontiguousarray(
        np.concatenate([Wq, Wk], axis=1).reshape(ND, 128, 128)
    )
    wv = np.ascontiguousarray(Wv.reshape(ND, 128, HD))
    ident = np.eye(64, dtype=np.float32)
    mask = np.triu(np.ones((128, 128), dtype=np.float32))
    zeros = np.zeros((64, T), dtype=np.float32)
    onescol = np.ones((128, NJ), dtype=np.float32)

    in_maps = []
    for c in range(NCORES):
        xt = np.ascontiguousarray(
            x[BPC * c:BPC * (c + 1)].transpose(0, 2, 1)
        )
        in_maps.append(
            {
                "xt": xt,
                "wqk": wqk,
                "wv": wv,
                "ident": ident,
                "mask": mask,
                "zeros": zeros,
                "onescol": onescol,
            }
        )

    kwargs = dict(_trace_kwargs or {})
    res = run_bass_kernel_spmd(
        nc, in_maps, list(range(NCORES)), trace=_trace, **kwargs
    )

    out = np.empty((B, T, HD), dtype=np.float32)
    for c in range(NCORES):
        ot = res.results[c]["ot"]  # [BPC, HD, T]
        out[BPC * c:BPC * (c + 1)] = ot.transpose(0, 2, 1)
    if _trace:
        _cache["last_results"] = res
    return out


# revision 18
# speedup vs baseline: 1.3225x; 1.3225x over previous
"""Causal single-head attention on 8 Trainium2 NeuronCores.

Reference computation (per batch b of 16):
    q = x @ Wq; k = x @ Wk; v = x @ Wv        # x [2048, 512], W* [512, 64]
    out = softmax_causal(q @ k.T / 8) @ v     # out [2048, 64]

Sharding: data-parallel over batch, 2 batches per core, weights replicated.

Per-core kernel (batch-local b in {0,1}), all matmuls in float32r
(TF32-like, full PE rate at N>=256):
  - host supplies xT = x[b].T so the D-contraction sits on partitions
  - qT/kT: psum[0:64]=qT, psum[64:128]=kT via packed lhsT [Wq|Wk]
  - kT copied to partitions 0-63 via SBUF->SBUF DMA (matmul operands
    must share partitions)
  - vT via Wv-stationary matmuls, then PE-transposed to v natural and
    packed as v1[., j, .] = [v_j | 1] (the ones column makes the PV
    matmul emit the softmax denominator for free)
  - scores TRANSPOSED: ST[k, q] = kT_j.T @ qT_Q -> psum, so softmax's
    denominator is a partition-dim sum that the PV matmul computes,
    and p~ = exp(ST) feeds the PV matmul with no transpose
  - exp on ACT straight out of psum in [128, 1024] chunks (2 k-blocks)
  - causal: k-blocks above the diagonal skipped; diagonal blocks get a
    triangular mask multiply and suffix-sliced matmuls
  - oT[65, 512] accumulates [v|1].T @ p~ over k-blocks in psum; row 64
    is the denominator l; out = oT[0:64] * broadcast(1/l) where the
    broadcast across partitions is a K=1 matmul against a ones column
  - output written transposed [2, 64, 2048]; host transposes back
"""

import sys

sys.path.insert(0, "/opt/trn_rl_repo")

import numpy as np

B, T, D, HD = 16, 2048, 512, 64
NCORES = 8
BPC = B // NCORES          # batches per core
NQ = T // 512              # 512-wide q chunks per batch
NJ = T // 128              # 128-wide k blocks per batch
ND = D // 128              # 128-deep contraction tiles

_cache = {}


def _build_nc():
    import concourse.bacc as bacc
    import concourse.mybir as mybir
    import concourse.tile as tile

    F32 = mybir.dt.float32
    F32R = mybir.dt.float32r
    AF = mybir.ActivationFunctionType

    nc = bacc.Bacc("TRN2", target_bir_lowering=False, debug=False)

    xt_d = nc.dram_tensor("xt", [BPC, D, T], F32, kind="ExternalInput")
    wqk_d = nc.dram_tensor("wqk", [ND, 128, 128], F32, kind="ExternalInput")
    wv_d = nc.dram_tensor("wv", [ND, 128, HD], F32, kind="ExternalInput")
    ident_d = nc.dram_tensor("ident", [64, 64], F32, kind="ExternalInput")
    mask_d = nc.dram_tensor("mask", [128, 128], F32, kind="ExternalInput")
    zeros_d = nc.dram_tensor("zeros", [64, T], F32, kind="ExternalInput")
    onescol_d = nc.dram_tensor("onescol", [128, NJ], F32, kind="ExternalInput")
    ot_d = nc.dram_tensor("ot", [BPC, HD, T], F32, kind="ExternalOutput")

    with tile.TileContext(nc) as tc:
        with (
            tc.tile_pool(name="const", bufs=1) as cpool,
            tc.tile_pool(name="xt", bufs=1) as xtpool,
            tc.tile_pool(name="qk", bufs=2) as qkpool,
            tc.tile_pool(name="klo", bufs=2) as klopool,
            tc.tile_pool(name="vt", bufs=2) as vtpool,
            tc.tile_pool(name="v1", bufs=2) as v1pool,
            tc.tile_pool(name="pt", bufs=3) as ptpool,
            tc.tile_pool(name="outp", bufs=2) as outpool,
            tc.tile_pool(name="rec", bufs=2) as recpool,
            tc.tile_pool(name="st", bufs=2, space="PSUM") as stpool,
            tc.tile_pool(name="otp", bufs=2, space="PSUM") as otpool,
            tc.tile_pool(name="aux", bufs=2, space="PSUM") as auxpool,
        ):
            # ---- per-batch persistent tiles; zero K-pad rows for klo,
            # split across DMA queues and issued first (single-queue DMA is
            # ~25 GB/s; latency matters) ----
            _qkpools = (qkpool, klopool, v1pool, vtpool)
            PRE = {}
            for b in range(BPC):
                PRE[b] = (
                    qkpool.tile([128, T], F32R, tag="qk", name=f"qk{b}"),
                    klopool.tile([128, T], F32R, tag="klo", name=f"klo{b}"),
                    v1pool.tile([128, NJ, HD + 1], F32R, tag="v1", name=f"v1{b}"),
                    vtpool.tile([64, T], F32, tag="vt", name=f"vt{b}"),
                )
            for b in range(BPC):
                for z8 in range(8):
                    nc.sync.dma_start(
                        PRE[b][1][64 + 8 * z8:64 + 8 * (z8 + 1), :],
                        zeros_d[8 * z8:8 * (z8 + 1), :].bitcast(F32R),
                    )

            # ---- constants / weights ----

            ident = cpool.tile([64, 64], F32, tag="ident")
            nc.sync.dma_start(ident[:], ident_d[:])
            mask = cpool.tile([128, 128], F32, tag="mask")
            nc.sync.dma_start(mask[:], mask_d[:])
            onescol = cpool.tile([128, NJ], F32, tag="onescol")
            nc.sync.dma_start(onescol[:], onescol_d[:])
            wqk = []
            for d in range(ND):
                w = cpool.tile([128, 128], F32R, tag=f"wqk{d}", name=f"wqk{d}")
                nc.sync.dma_start(w[:], wqk_d[d].bitcast(F32R))
                wqk.append(w)
            wv = []
            for d in range(ND):
                w = cpool.tile([128, HD], F32R, tag=f"wv{d}", name=f"wv{d}")
                nc.sync.dma_start(w[:], wv_d[d].bitcast(F32R))
                wv.append(w)

            # warm the exp table set on ACT while projections run
            scratch = cpool.tile([1, 1], F32, tag="scratch")
            nc.scalar.activation(scratch[:], mask[0:1, 0:1], AF.Exp)


            xts = {}   # (b, d) -> tile [128, T] F32R
            qks, klos, v1s, vts = {}, {}, {}, {}
            for b in range(BPC):
                qks[b], klos[b], v1s[b], vts[b] = PRE[b]
                nc.vector.tensor_copy(
                    v1s[b][:, :, HD:HD + 1],
                    onescol[:].rearrange("p (a c) -> p a c", c=1),
                )

            def emit_xt_dmas(b, Q):
                for d in range(ND):
                    if (b, d) not in xts:
                        xts[(b, d)] = xtpool.tile(
                            [128, T], F32R, tag=f"xt{b}{d}", name=f"xt{b}{d}"
                        )
                    s = slice(512 * Q, 512 * (Q + 1))
                    for r2 in range(2):
                        nc.sync.dma_start(
                            xts[(b, d)][64 * r2:64 * (r2 + 1), s],
                            xt_d[b, 128 * d + 64 * r2:128 * d + 64 * (r2 + 1),
                                 s].bitcast(F32R),
                        )

            def emit_proj_q(b, Q):
                """Everything attention chunk (b, Q) will need from tokens
                [512Q, 512Q+512): qT/kT, k shifted+zero-padded, v transposed."""
                s = slice(512 * Q, 512 * (Q + 1))
                qk, klo, v1, vt = qks[b], klos[b], v1s[b], vts[b]

                p = auxpool.tile([128, 512], F32, tag="aux", name="pqk")
                for d in range(ND):
                    nc.tensor.matmul(
                        p[:], wqk[d][:], xts[(b, d)][:, s],
                        start=(d == 0), stop=(d == ND - 1),
                    )
                nc.vector.tensor_copy(qk[:, s], p[:])
                # kT to partitions 0:64 (rows 64:128 are the zero K-pad);
                # partition-split 4 ways so the shift spreads across queues
                for p4 in range(4):
                    nc.sync.dma_start(
                        klo[16 * p4:16 * (p4 + 1), s],
                        qk[64 + 16 * p4:64 + 16 * (p4 + 1), s],
                    )

                pv_ = auxpool.tile([64, 512], F32, tag="aux", name="pvt")
                for d in range(ND):
                    nc.tensor.matmul(
                        pv_[:], wv[d][:], xts[(b, d)][:, s],
                        start=(d == 0), stop=(d == ND - 1),
                    )
                nc.vector.tensor_copy(vt[:, s], pv_[:])
                for t2 in range(2 * Q, 2 * Q + 2):
                    p2 = auxpool.tile([128, 128], F32, tag="aux", name="ptr")
                    for tt in range(2):
                        nc.tensor.transpose(
                            p2[:, 64 * tt:64 * (tt + 1)],
                            vt[:, 128 * (2 * t2 + tt):128 * (2 * t2 + tt + 1)],
                            ident[:],
                        )
                    nc.vector.tensor_copy(
                        v1[:, 2 * t2:2 * t2 + 2, 0:HD],
                        p2[:].rearrange("p (a c) -> p a c", a=2),
                    )

            def emit_attn_q(b, Q):
                """One query chunk: all causal k-blocks, paired into
                [128,1024] psum chunks; PV skewed one chunk behind ST."""
                qk, klo, v1 = qks[b], klos[b], v1s[b]
                pot = otpool.tile([65, 512], F32, tag="ot", name="pot")
                njb = 4 * (Q + 1)          # causal k-blocks for this chunk
                jlast = njb - 1
                chunks = [(2 * g, 2 * g + 1) for g in range(njb // 2)]
                pending = None

                def emit_pv(p_tile, js):
                    for idx, j in enumerate(js):
                        w0 = 128 * (j - 4 * Q) if j >= 4 * Q else 0
                        nc.tensor.matmul(
                            pot[:, w0:512],
                            v1[:, j, :],
                            p_tile[:, 512 * idx + w0:512 * (idx + 1)],
                            start=(j == 0),
                            stop=(j == jlast),
                        )

                for js in chunks:
                    pst = stpool.tile([128, 1024], F32, tag="st", name="pst")
                    for idx, j in enumerate(js):
                        w0 = 128 * (j - 4 * Q) if j >= 4 * Q else 0
                        nc.tensor.matmul(
                            pst[:, 512 * idx + w0:512 * (idx + 1)],
                            klo[:, 128 * j:128 * (j + 1)],
                            qk[:, 512 * Q + w0:512 * (Q + 1)],
                            start=True, stop=True,
                        )
                    ptil = ptpool.tile([128, 1024], F32R, tag="pt", name="ptil")
                    nc.scalar.activation(
                        ptil[:], pst[:], AF.Exp, scale=1.0 / np.sqrt(HD)
                    )
                    for idx, j in enumerate(js):
                        if j >= 4 * Q:  # diagonal block: triangular mask
                            w0 = 128 * (j - 4 * Q)
                            win = slice(512 * idx + w0, 512 * idx + w0 + 128)
                            nc.vector.tensor_mul(
                                ptil[:, win], ptil[:, win].bitcast(F32), mask[:]
                            )
                    if pending is not None:
                        emit_pv(*pending)
                    pending = (ptil, js)
                emit_pv(*pending)

                # normalize: out = oT[0:64] * broadcast(1 / l), PE-free
                lsb = recpool.tile([1, 512], F32, tag="lsb", name="lsb")
                nc.vector.tensor_copy(lsb[:], pot[64:65, :])
                rsb = recpool.tile([1, 512], F32, tag="rsb", name="rsb")
                nc.vector.reciprocal_approx_fast(rsb[:], lsb[:])
                rcb = outpool.tile([HD, 512], F32, tag="rcb", name="rcb")
                nc.gpsimd.partition_broadcast(rcb[:], rsb[:])
                osb = outpool.tile([HD, 512], F32, tag="out", name="osb")
                nc.vector.tensor_mul(osb[:], pot[0:HD, :], rcb[:])
                nc.sync.dma_start(ot_d[0 if b == 0 else 1, :, 512 * Q:512 * (Q + 1)], osb[:])

            # ---- emission schedule: projections feed attention per-Q ----
            for Q in range(NQ):
                emit_xt_dmas(0, Q)
            emit_proj_q(0, 0)
            emit_attn_q(0, 0)
            emit_proj_q(0, 1)
            emit_attn_q(0, 1)
            emit_proj_q(0, 2)
            for Q in range(NQ):
                emit_xt_dmas(1, Q)
            emit_attn_q(0, 2)
            emit_proj_q(0, 3)
            emit_attn_q(0, 3)
            emit_proj_q(1, 0)
            emit_attn_q(1, 0)
            emit_proj_q(1, 1)
            emit_attn_q(1, 1)
            emit_proj_q(1, 2)
            emit_attn_q(1, 2)
            emit_proj_q(1, 3)
            emit_attn_q(1, 3)

    nc.compile()
    return nc


def _get_nc():
    if "nc" not in _cache:
        _cache["nc"] = _build_nc()
    return _cache["nc"]


def kernel(x, Wq, Wk, Wv, _trace=False, _trace_kwargs=None):
    from concourse.bass_utils import run_bass_kernel_spmd

    x = np.asarray(x, dtype=np.float32)
    Wq = np.asarray(Wq, dtype=np.float32)
    Wk = np.asarray(Wk, dtype=np.float32)
    Wv = np.asarray(Wv, dtype=np.float32)

    nc = _get_nc()

    wqk = np.ascontiguousarray(
        np.concatenate([Wq, Wk], axis=1).reshape(ND, 128, 128)
    )
    wv = np.ascontiguousarray(Wv.reshape(ND, 128, HD))
    ident = np.eye(64, dtype=np.float32)
    mask = np.triu(np.ones((128, 128), dtype=np.float32))
    zeros = np.zeros((64, T), dtype=np.float32)
    onescol = np.ones((128, NJ), dtype=np.float32)

    in_maps = []
    for c in range(NCORES):
        xt = np.ascontiguousarray(
            x[BPC * c:BPC * (c + 1)].transpose(0, 2, 1)
        )
        in_maps.append(
            {
                "xt": xt,
                "wqk": wqk,
                "wv": wv,
                "ident": ident,
                "mask": mask,
                "zeros": zeros,
                "onescol": onescol,
            }
        )

    kwargs = dict(_trace_kwargs or {})
    res = run_bass_kernel_spmd(
        nc, in_maps, list(range(NCORES)), trace=_trace, **kwargs
    )

    out = np.empty((B, T, HD), dtype=np.float32)
    for c in range(NCORES):
        ot = res.results[c]["ot"]  # [BPC, HD, T]
        out[BPC * c:BPC * (c + 1)] = ot.transpose(0, 2, 1)
    if _trace:
        _cache["last_results"] = res
    return out


# revision 19
# speedup vs baseline: 1.4561x; 1.1011x over previous
"""Causal single-head attention on 8 Trainium2 NeuronCores.

Reference computation (per batch b of 16):
    q = x @ Wq; k = x @ Wk; v = x @ Wv        # x [2048, 512], W* [512, 64]
    out = softmax_causal(q @ k.T / 8) @ v     # out [2048, 64]

Sharding: data-parallel over batch, 2 batches per core, weights replicated.

Per-core kernel (batch-local b in {0,1}), all matmuls in float32r
(TF32-like, full PE rate at N>=256):
  - host supplies xT = x[b].T so the D-contraction sits on partitions
  - qT/kT: psum[0:64]=qT, psum[64:128]=kT via packed lhsT [Wq|Wk]
  - kT copied to partitions 0-63 via SBUF->SBUF DMA (matmul operands
    must share partitions)
  - vT via Wv-stationary matmuls, then PE-transposed to v natural and
    packed as v1[., j, .] = [v_j | 1] (the ones column makes the PV
    matmul emit the softmax denominator for free)
  - scores TRANSPOSED: ST[k, q] = kT_j.T @ qT_Q -> psum, so softmax's
    denominator is a partition-dim sum that the PV matmul computes,
    and p~ = exp(ST) feeds the PV matmul with no transpose
  - exp on ACT straight out of psum in [128, 1024] chunks (2 k-blocks)
  - causal: k-blocks above the diagonal skipped; diagonal blocks get a
    triangular mask multiply and suffix-sliced matmuls
  - oT[65, 512] accumulates [v|1].T @ p~ over k-blocks in psum; row 64
    is the denominator l; out = oT[0:64] * broadcast(1/l) where the
    broadcast across partitions is a K=1 matmul against a ones column
  - output written transposed [2, 64, 2048]; host transposes back
"""

import sys

sys.path.insert(0, "/opt/trn_rl_repo")

import numpy as np

B, T, D, HD = 16, 2048, 512, 64
NCORES = 8
BPC = B // NCORES          # batches per core
NQ = T // 512              # 512-wide q chunks per batch
NJ = T // 128              # 128-wide k blocks per batch
ND = D // 128              # 128-deep contraction tiles

_cache = {}


def _build_nc():
    import concourse.bacc as bacc
    import concourse.mybir as mybir
    import concourse.tile as tile

    F32 = mybir.dt.float32
    F32R = mybir.dt.float32r
    AF = mybir.ActivationFunctionType

    nc = bacc.Bacc("TRN2", target_bir_lowering=False, debug=False)

    xt_d = nc.dram_tensor("xt", [BPC, D, T], F32, kind="ExternalInput")
    wqk_d = nc.dram_tensor("wqk", [ND, 128, 128], F32, kind="ExternalInput")
    wv_d = nc.dram_tensor("wv", [ND, 128, HD], F32, kind="ExternalInput")
    ident_d = nc.dram_tensor("ident", [64, 64], F32, kind="ExternalInput")
    mask_d = nc.dram_tensor("mask", [128, 128], F32, kind="ExternalInput")
    zeros_d = nc.dram_tensor("zeros", [64, T], F32, kind="ExternalInput")
    onescol_d = nc.dram_tensor("onescol", [128, NJ], F32, kind="ExternalInput")
    ot_d = nc.dram_tensor("ot", [BPC, HD, T], F32, kind="ExternalOutput")

    with tile.TileContext(nc) as tc:
        with (
            tc.tile_pool(name="const", bufs=1) as cpool,
            tc.tile_pool(name="xt", bufs=1) as xtpool,
            tc.tile_pool(name="qk", bufs=2) as qkpool,
            tc.tile_pool(name="klo", bufs=2) as klopool,
            tc.tile_pool(name="vt", bufs=2) as vtpool,
            tc.tile_pool(name="v1", bufs=2) as v1pool,
            tc.tile_pool(name="pt", bufs=3) as ptpool,
            tc.tile_pool(name="outp", bufs=2) as outpool,
            tc.tile_pool(name="rec", bufs=2) as recpool,
            tc.tile_pool(name="st", bufs=2, space="PSUM") as stpool,
            tc.tile_pool(name="otp", bufs=2, space="PSUM") as otpool,
            tc.tile_pool(name="aux", bufs=2, space="PSUM") as auxpool,
        ):
            # ---- per-batch persistent tiles; zero K-pad rows for klo,
            # split across DMA queues and issued first (single-queue DMA is
            # ~25 GB/s; latency matters) ----
            _qkpools = (qkpool, klopool, v1pool, vtpool)
            PRE = {}
            for b in range(BPC):
                PRE[b] = (
                    qkpool.tile([128, T], F32R, tag="qk", name=f"qk{b}"),
                    klopool.tile([128, T], F32R, tag="klo", name=f"klo{b}"),
                    v1pool.tile([128, NJ, HD + 1], F32R, tag="v1", name=f"v1{b}"),
                    vtpool.tile([64, T], F32, tag="vt", name=f"vt{b}"),
                )
            for b in range(BPC):
                for z8 in range(8):
                    nc.sync.dma_start(
                        PRE[b][1][64 + 8 * z8:64 + 8 * (z8 + 1), :],
                        zeros_d[8 * z8:8 * (z8 + 1), :].bitcast(F32R),
                    )

            # ---- constants / weights ----

            ident = cpool.tile([64, 64], F32, tag="ident")
            nc.sync.dma_start(ident[:], ident_d[:])
            mask = cpool.tile([128, 128], F32, tag="mask")
            nc.sync.dma_start(mask[:], mask_d[:])
            onescol = cpool.tile([128, NJ], F32, tag="onescol")
            nc.sync.dma_start(onescol[:], onescol_d[:])
            wqk = []
            for d in range(ND):
                w = cpool.tile([128, 128], F32R, tag=f"wqk{d}", name=f"wqk{d}")
                nc.sync.dma_start(w[:], wqk_d[d].bitcast(F32R))
                wqk.append(w)
            wv = []
            for d in range(ND):
                w = cpool.tile([128, HD], F32R, tag=f"wv{d}", name=f"wv{d}")
                nc.sync.dma_start(w[:], wv_d[d].bitcast(F32R))
                wv.append(w)

            # warm the exp table set on ACT while projections run
            scratch = cpool.tile([1, 1], F32, tag="scratch")
            nc.scalar.activation(scratch[:], mask[0:1, 0:1], AF.Exp)


            xts = {}   # (b, d) -> tile [128, T] F32R
            qks, klos, v1s, vts = {}, {}, {}, {}
            for b in range(BPC):
                qks[b], klos[b], v1s[b], vts[b] = PRE[b]
                nc.vector.tensor_copy(
                    v1s[b][:, :, HD:HD + 1],
                    onescol[:].rearrange("p (a c) -> p a c", c=1),
                )

            def emit_xt_dmas(b, Q):
                for d in range(ND):
                    if (b, d) not in xts:
                        xts[(b, d)] = xtpool.tile(
                            [128, T], F32R, tag=f"xt{b}{d}", name=f"xt{b}{d}"
                        )
                    s = slice(512 * Q, 512 * (Q + 1))
                    nc.sync.dma_start(
                        xts[(b, d)][:, s],
                        xt_d[b, 128 * d:128 * (d + 1), s].bitcast(F32R),
                    )

            def emit_proj_q(b, Q):
                """Everything attention chunk (b, Q) will need from tokens
                [512Q, 512Q+512): qT/kT, k shifted+zero-padded, v transposed."""
                s = slice(512 * Q, 512 * (Q + 1))
                qk, klo, v1, vt = qks[b], klos[b], v1s[b], vts[b]

                p = auxpool.tile([128, 512], F32, tag="aux", name="pqk")
                for d in range(ND):
                    nc.tensor.matmul(
                        p[:], wqk[d][:], xts[(b, d)][:, s],
                        start=(d == 0), stop=(d == ND - 1),
                    )
                nc.vector.tensor_copy(qk[:, s], p[:])
                # kT to partitions 0:64 (rows 64:128 are the zero K-pad);
                # partition-split 4 ways so the shift spreads across queues
                for p4 in range(4):
                    nc.sync.dma_start(
                        klo[16 * p4:16 * (p4 + 1), s],
                        qk[64 + 16 * p4:64 + 16 * (p4 + 1), s],
                    )

                pv_ = auxpool.tile([64, 512], F32, tag="aux", name="pvt")
                for d in range(ND):
                    nc.tensor.matmul(
                        pv_[:], wv[d][:], xts[(b, d)][:, s],
                        start=(d == 0), stop=(d == ND - 1),
                    )
                nc.vector.tensor_copy(vt[:, s], pv_[:])
                for t2 in range(2 * Q, 2 * Q + 2):
                    p2 = auxpool.tile([128, 128], F32, tag="aux", name="ptr")
                    for tt in range(2):
                        nc.tensor.transpose(
                            p2[:, 64 * tt:64 * (tt + 1)],
                            vt[:, 128 * (2 * t2 + tt):128 * (2 * t2 + tt + 1)],
                            ident[:],
                        )
                    nc.vector.tensor_copy(
                        v1[:, 2 * t2:2 * t2 + 2, 0:HD],
                        p2[:].rearrange("p (a c) -> p a c", a=2),
                    )

            def emit_attn_q(b, Q):
                """One query chunk: all causal k-blocks, paired into
                [128,1024] psum chunks; PV skewed one chunk behind ST."""
                qk, klo, v1 = qks[b], klos[b], v1s[b]
                pot = otpool.tile([65, 512], F32, tag="ot", name="pot")
                njb = 4 * (Q + 1)          # causal k-blocks for this chunk
                jlast = njb - 1
                chunks = [(2 * g, 2 * g + 1) for g in range(njb // 2)]
                pending = None

                def emit_pv(p_tile, js):
                    for idx, j in enumerate(js):
                        w0 = 128 * (j - 4 * Q) if j >= 4 * Q else 0
                        nc.tensor.matmul(
                            pot[:, w0:512],
                            v1[:, j, :],
                            p_tile[:, 512 * idx + w0:512 * (idx + 1)],
                            start=(j == 0),
                            stop=(j == jlast),
                        )

                for js in chunks:
                    pst = stpool.tile([128, 1024], F32, tag="st", name="pst")
                    for idx, j in enumerate(js):
                        w0 = 128 * (j - 4 * Q) if j >= 4 * Q else 0
                        nc.tensor.matmul(
                            pst[:, 512 * idx + w0:512 * (idx + 1)],
                            klo[:, 128 * j:128 * (j + 1)],
                            qk[:, 512 * Q + w0:512 * (Q + 1)],
                            start=True, stop=True,
                        )
                    ptil = ptpool.tile([128, 1024], F32R, tag="pt", name="ptil")
                    nc.scalar.activation(
                        ptil[:], pst[:], AF.Exp, scale=1.0 / np.sqrt(HD)
                    )
                    for idx, j in enumerate(js):
                        if j >= 4 * Q:  # diagonal block: triangular mask
                            w0 = 128 * (j - 4 * Q)
                            win = slice(512 * idx + w0, 512 * idx + w0 + 128)
                            nc.vector.tensor_mul(
                                ptil[:, win], ptil[:, win].bitcast(F32), mask[:]
                            )
                    if pending is not None:
                        emit_pv(*pending)
                    pending = (ptil, js)
                emit_pv(*pending)

                # normalize: out = oT[0:64] * broadcast(1 / l), PE-free
                lsb = recpool.tile([1, 512], F32, tag="lsb", name="lsb")
                nc.vector.tensor_copy(lsb[:], pot[64:65, :])
                rsb = recpool.tile([1, 512], F32, tag="rsb", name="rsb")
                nc.vector.reciprocal_approx_fast(rsb[:], lsb[:])
                rcb = outpool.tile([HD, 512], F32, tag="rcb", name="rcb")
                nc.gpsimd.partition_broadcast(rcb[:], rsb[:])
                osb = outpool.tile([HD, 512], F32, tag="out", name="osb")
                nc.vector.tensor_mul(osb[:], pot[0:HD, :], rcb[:])
                nc.sync.dma_start(ot_d[0 if b == 0 else 1, :, 512 * Q:512 * (Q + 1)], osb[:])

            # ---- emission schedule: projections feed attention per-Q ----
            for Q in range(NQ):
                emit_xt_dmas(0, Q)
            emit_proj_q(0, 0)
            emit_attn_q(0, 0)
            emit_proj_q(0, 1)
            emit_attn_q(0, 1)
            emit_proj_q(0, 2)
            for Q in range(NQ):
                emit_xt_dmas(1, Q)
            emit_attn_q(0, 2)
            emit_proj_q(0, 3)
            emit_attn_q(0, 3)
            emit_proj_q(1, 0)
            emit_attn_q(1, 0)
            emit_proj_q(1, 1)
            emit_attn_q(1, 1)
            emit_proj_q(1, 2)
            emit_attn_q(1, 2)
            emit_proj_q(1, 3)
            emit_attn_q(1, 3)

    nc.compile()
    return nc


def _get_nc():
    if "nc" not in _cache:
        _cache["nc"] = _build_nc()
    return _cache["nc"]


def kernel(x, Wq, Wk, Wv, _trace=False, _trace_kwargs=None):
    from concourse.bass_utils import run_bass_kernel_spmd

    x = np.asarray(x, dtype=np.float32)
    Wq = np.asarray(Wq, dtype=np.float32)
    Wk = np.asarray(Wk, dtype=np.float32)
    Wv = np.asarray(Wv, dtype=np.float32)

    nc = _get_nc()

    wqk = np.ascontiguousarray(
        np.concatenate([Wq, Wk], axis=1).reshape(ND, 128, 128)
    )
    wv = np.ascontiguousarray(Wv.reshape(ND, 128, HD))
    ident = np.eye(64, dtype=np.float32)
    mask = np.triu(np.ones((128, 128), dtype=np.float32))
    zeros = np.zeros((64, T), dtype=np.float32)
    onescol = np.ones((128, NJ), dtype=np.float32)

    in_maps = []
    for c in range(NCORES):
        xt = np.ascontiguousarray(
            x[BPC * c:BPC * (c + 1)].transpose(0, 2, 1)
        )
        in_maps.append(
            {
                "xt": xt,
                "wqk": wqk,
                "wv": wv,
                "ident": ident,
                "mask": mask,
                "zeros": zeros,
                "onescol": onescol,
            }
        )

    kwargs = dict(_trace_kwargs or {})
    res = run_bass_kernel_spmd(
        nc, in_maps, list(range(NCORES)), trace=_trace, **kwargs
    )

    out = np.empty((B, T, HD), dtype=np.float32)
    for c in range(NCORES):
        ot = res.results[c]["ot"]  # [BPC, HD, T]
        out[BPC * c:BPC * (c + 1)] = ot.transpose(0, 2, 1)
    if _trace:
        _cache["last_results"] = res
    return out


# revision 20
# speedup vs baseline: 1.7095x; 1.1740x over previous
"""Causal single-head attention on 8 Trainium2 NeuronCores.

Reference computation (per batch b of 16):
    q = x @ Wq; k = x @ Wk; v = x @ Wv        # x [2048, 512], W* [512, 64]
    out = softmax_causal(q @ k.T / 8) @ v     # out [2048, 64]

Sharding: data-parallel over batch, 2 batches per core, weights replicated.

Per-core kernel (batch-local b in {0,1}), all matmuls in float32r
(TF32-like, full PE rate at N>=256):
  - host supplies xT = x[b].T so the D-contraction sits on partitions
  - qT/kT: psum[0:64]=qT, psum[64:128]=kT via packed lhsT [Wq|Wk]
  - kT copied to partitions 0-63 via SBUF->SBUF DMA (matmul operands
    must share partitions)
  - vT via Wv-stationary matmuls, then PE-transposed to v natural and
    packed as v1[., j, .] = [v_j | 1] (the ones column makes the PV
    matmul emit the softmax denominator for free)
  - scores TRANSPOSED: ST[k, q] = kT_j.T @ qT_Q -> psum, so softmax's
    denominator is a partition-dim sum that the PV matmul computes,
    and p~ = exp(ST) feeds the PV matmul with no transpose
  - exp on ACT straight out of psum in [128, 1024] chunks (2 k-blocks)
  - causal: k-blocks above the diagonal skipped; diagonal blocks get a
    triangular mask multiply and suffix-sliced matmuls
  - oT[65, 512] accumulates [v|1].T @ p~ over k-blocks in psum; row 64
    is the denominator l; out = oT[0:64] * broadcast(1/l) where the
    broadcast across partitions is a K=1 matmul against a ones column
  - output written transposed [2, 64, 2048]; host transposes back
"""

import sys

sys.path.insert(0, "/opt/trn_rl_repo")

import numpy as np

B, T, D, HD = 16, 2048, 512, 64
NCORES = 8
BPC = B // NCORES          # batches per core
NQ = T // 512              # 512-wide q chunks per batch
NJ = T // 128              # 128-wide k blocks per batch
ND = D // 128              # 128-deep contraction tiles

_cache = {}


def _build_nc():
    import concourse.bacc as bacc
    import concourse.mybir as mybir
    import concourse.tile as tile

    F32 = mybir.dt.float32
    F32R = mybir.dt.float32r
    AF = mybir.ActivationFunctionType

    nc = bacc.Bacc("TRN2", target_bir_lowering=False, debug=False)

    xt_d = nc.dram_tensor("xt", [BPC, D, T], F32, kind="ExternalInput")
    wqk_d = nc.dram_tensor("wqk", [ND, 128, 128], F32, kind="ExternalInput")
    wv_d = nc.dram_tensor("wv", [ND, 128, HD], F32, kind="ExternalInput")
    ident_d = nc.dram_tensor("ident", [64, 64], F32, kind="ExternalInput")
    mask_d = nc.dram_tensor("mask", [128, 128], F32, kind="ExternalInput")
    zeros_d = nc.dram_tensor("zeros", [64, T], F32, kind="ExternalInput")
    onescol_d = nc.dram_tensor("onescol", [128, NJ], F32, kind="ExternalInput")
    ot_d = nc.dram_tensor("ot", [BPC, HD, T], F32, kind="ExternalOutput")

    with tile.TileContext(nc) as tc:
        with (
            tc.tile_pool(name="const", bufs=1) as cpool,
            tc.tile_pool(name="xt", bufs=1) as xtpool,
            tc.tile_pool(name="qk", bufs=2) as qkpool,
            tc.tile_pool(name="klo", bufs=2) as klopool,
            tc.tile_pool(name="vt", bufs=2) as vtpool,
            tc.tile_pool(name="v1", bufs=2) as v1pool,
            tc.tile_pool(name="pt", bufs=3) as ptpool,
            tc.tile_pool(name="outp", bufs=2) as outpool,
            tc.tile_pool(name="rec", bufs=2) as recpool,
            tc.tile_pool(name="st", bufs=2, space="PSUM") as stpool,
            tc.tile_pool(name="otp", bufs=2, space="PSUM") as otpool,
            tc.tile_pool(name="aux", bufs=2, space="PSUM") as auxpool,
        ):
            # ---- constants / weights ----

            ident = cpool.tile([64, 64], F32, tag="ident")
            nc.sync.dma_start(ident[:], ident_d[:])
            mask = cpool.tile([128, 128], F32, tag="mask")
            nc.sync.dma_start(mask[:], mask_d[:])
            onescol = cpool.tile([128, NJ], F32, tag="onescol")
            nc.sync.dma_start(onescol[:], onescol_d[:])
            wqk = []
            for d in range(ND):
                w = cpool.tile([128, 128], F32R, tag=f"wqk{d}", name=f"wqk{d}")
                nc.sync.dma_start(w[:], wqk_d[d].bitcast(F32R))
                wqk.append(w)
            wv = []
            for d in range(ND):
                w = cpool.tile([128, HD], F32R, tag=f"wv{d}", name=f"wv{d}")
                nc.sync.dma_start(w[:], wv_d[d].bitcast(F32R))
                wv.append(w)

            # warm the exp table set on ACT while projections run
            scratch = cpool.tile([1, 1], F32, tag="scratch")
            nc.scalar.activation(scratch[:], mask[0:1, 0:1], AF.Exp)


            xts = {}   # (b, d) -> tile [128, T] F32R
            qks, klos, v1s, vts = {}, {}, {}, {}
            for b in range(BPC):
                qks[b] = qkpool.tile([128, T], F32R, tag="qk", name=f"qk{b}")
                klos[b] = klopool.tile([128, T], F32R, tag="klo", name=f"klo{b}")
                v1s[b] = v1pool.tile([128, NJ, HD + 1], F32R, tag="v1", name=f"v1{b}")
                vts[b] = vtpool.tile([64, T], F32, tag="vt", name=f"vt{b}")
                nc.sync.dma_start(klos[b][64:128, :], zeros_d[:].bitcast(F32R))
                nc.vector.tensor_copy(
                    v1s[b][:, :, HD:HD + 1],
                    onescol[:].rearrange("p (a c) -> p a c", c=1),
                )

            def emit_xt_dmas(b, Q):
                for d in range(ND):
                    if (b, d) not in xts:
                        xts[(b, d)] = xtpool.tile(
                            [128, T], F32R, tag=f"xt{b}{d}", name=f"xt{b}{d}"
                        )
                    s = slice(512 * Q, 512 * (Q + 1))
                    nc.sync.dma_start(
                        xts[(b, d)][:, s],
                        xt_d[b, 128 * d:128 * (d + 1), s].bitcast(F32R),
                    )

            def emit_proj_q(b, Q):
                """Everything attention chunk (b, Q) will need from tokens
                [512Q, 512Q+512): qT/kT, k shifted+zero-padded, v transposed."""
                s = slice(512 * Q, 512 * (Q + 1))
                qk, klo, v1, vt = qks[b], klos[b], v1s[b], vts[b]

                p = auxpool.tile([128, 512], F32, tag="aux", name="pqk")
                for d in range(ND):
                    nc.tensor.matmul(
                        p[:], wqk[d][:], xts[(b, d)][:, s],
                        start=(d == 0), stop=(d == ND - 1),
                    )
                nc.vector.tensor_copy(qk[:, s], p[:])
                # kT to partitions 0:64 (rows 64:128 are the zero K-pad);
                # 4 DMAs so the shift spreads across queues
                for j4 in range(4):
                    s4 = slice(512 * Q + 128 * j4, 512 * Q + 128 * (j4 + 1))
                    nc.sync.dma_start(klo[0:64, s4], qk[64:128, s4])

                pv_ = auxpool.tile([64, 512], F32, tag="aux", name="pvt")
                for d in range(ND):
                    nc.tensor.matmul(
                        pv_[:], wv[d][:], xts[(b, d)][:, s],
                        start=(d == 0), stop=(d == ND - 1),
                    )
                nc.vector.tensor_copy(vt[:, s], pv_[:])
                for t2 in range(2 * Q, 2 * Q + 2):
                    p2 = auxpool.tile([128, 128], F32, tag="aux", name="ptr")
                    for tt in range(2):
                        nc.tensor.transpose(
                            p2[:, 64 * tt:64 * (tt + 1)],
                            vt[:, 128 * (2 * t2 + tt):128 * (2 * t2 + tt + 1)],
                            ident[:],
                        )
                    nc.vector.tensor_copy(
                        v1[:, 2 * t2:2 * t2 + 2, 0:HD],
                        p2[:].rearrange("p (a c) -> p a c", a=2),
                    )

            def emit_attn_q(b, Q):
                """One query chunk: all causal k-blocks, paired into
                [128,1024] psum chunks; PV skewed one chunk behind ST."""
                qk, klo, v1 = qks[b], klos[b], v1s[b]
                pot = otpool.tile([65, 512], F32, tag="ot", name="pot")
                njb = 4 * (Q + 1)          # causal k-blocks for this chunk
                jlast = njb - 1
                chunks = [(2 * g, 2 * g + 1) for g in range(njb // 2)]
                pending = None

                def emit_pv(p_tile, js):
                    for idx, j in enumerate(js):
                        w0 = 128 * (j - 4 * Q) if j >= 4 * Q else 0
                        nc.tensor.matmul(
                            pot[:, w0:512],
                            v1[:, j, :],
                            p_tile[:, 512 * idx + w0:512 * (idx + 1)],
                            start=(j == 0),
                            stop=(j == jlast),
                        )

                for js in chunks:
                    pst = stpool.tile([128, 1024], F32, tag="st", name="pst")
                    for idx, j in enumerate(js):
                        w0 = 128 * (j - 4 * Q) if j >= 4 * Q else 0
                        nc.tensor.matmul(
                            pst[:, 512 * idx + w0:512 * (idx + 1)],
                            klo[:, 128 * j:128 * (j + 1)],
                            qk[:, 512 * Q + w0:512 * (Q + 1)],
                            start=True, stop=True,
                        )
                    ptil = ptpool.tile([128, 1024], F32R, tag="pt", name="ptil")
                    nc.scalar.activation(
                        ptil[:], pst[:], AF.Exp, scale=1.0 / np.sqrt(HD)
                    )
                    for idx, j in enumerate(js):
                        if j >= 4 * Q:  # diagonal block: triangular mask
                            w0 = 128 * (j - 4 * Q)
                            win = slice(512 * idx + w0, 512 * idx + w0 + 128)
                            nc.vector.tensor_mul(
                                ptil[:, win], ptil[:, win].bitcast(F32), mask[:]
                            )
                    if pending is not None:
                        emit_pv(*pending)
                    pending = (ptil, js)
                emit_pv(*pending)

                # normalize: out = oT[0:64] * broadcast(1 / l), PE-free
                lsb = recpool.tile([1, 512], F32, tag="lsb", name="lsb")
                nc.vector.tensor_copy(lsb[:], pot[64:65, :])
                rsb = recpool.tile([1, 512], F32, tag="rsb", name="rsb")
                nc.vector.reciprocal_approx_fast(rsb[:], lsb[:])
                rcb = outpool.tile([HD, 512], F32, tag="rcb", name="rcb")
                nc.gpsimd.partition_broadcast(rcb[:], rsb[:])
                osb = outpool.tile([HD, 512], F32, tag="out", name="osb")
                nc.vector.tensor_mul(osb[:], pot[0:HD, :], rcb[:])
                nc.sync.dma_start(ot_d[0 if b == 0 else 1, :, 512 * Q:512 * (Q + 1)], osb[:])

            # ---- emission schedule: projections feed attention per-Q ----
            for Q in range(NQ):
                emit_xt_dmas(0, Q)
            emit_proj_q(0, 0)
            emit_attn_q(0, 0)
            emit_proj_q(0, 1)
            emit_attn_q(0, 1)
            emit_proj_q(0, 2)
            for Q in range(NQ):
                emit_xt_dmas(1, Q)
            emit_attn_q(0, 2)
            emit_proj_q(0, 3)
            emit_attn_q(0, 3)
            emit_proj_q(1, 0)
            emit_attn_q(1, 0)
            emit_proj_q(1, 1)
            emit_attn_q(1, 1)
            emit_proj_q(1, 2)
            emit_attn_q(1, 2)
            emit_proj_q(1, 3)
            emit_attn_q(1, 3)

    nc.compile()
    return nc


def _get_nc():
    if "nc" not in _cache:
        _cache["nc"] = _build_nc()
    return _cache["nc"]


def kernel(x, Wq, Wk, Wv, _trace=False, _trace_kwargs=None):
    from concourse.bass_utils import run_bass_kernel_spmd

    x = np.asarray(x, dtype=np.float32)
    Wq = np.asarray(Wq, dtype=np.float32)
    Wk = np.asarray(Wk, dtype=np.float32)
    Wv = np.asarray(Wv, dtype=np.float32)

    nc = _get_nc()

    wqk = np.ascontiguousarray(
        np.concatenate([Wq, Wk], axis=1).reshape(ND, 128, 128)
    )
    wv = np.ascontiguousarray(Wv.reshape(ND, 128, HD))
    ident = np.eye(64, dtype=np.float32)
    mask = np.triu(np.ones((128, 128), dtype=np.float32))
    zeros = np.zeros((64, T), dtype=np.float32)
    onescol = np.ones((128, NJ), dtype=np.float32)

    in_maps = []
    for c in range(NCORES):
        xt = np.ascontiguousarray(
            x[BPC * c:BPC * (c + 1)].transpose(0, 2, 1)
        )
        in_maps.append(
            {
                "xt": xt,
                "wqk": wqk,
                "wv": wv,
                "ident": ident,
                "mask": mask,
                "zeros": zeros,
                "onescol": onescol,
            }
        )

    kwargs = dict(_trace_kwargs or {})
    res = run_bass_kernel_spmd(
        nc, in_maps, list(range(NCORES)), trace=_trace, **kwargs
    )

    out = np.empty((B, T, HD), dtype=np.float32)
    for c in range(NCORES):
        ot = res.results[c]["ot"]  # [BPC, HD, T]
        out[BPC * c:BPC * (c + 1)] = ot.transpose(0, 2, 1)
    if _trace:
        _cache["last_results"] = res
    return out
